# revision 43
# baseline (speedup 1.0000x reference)
import sys
sys.path.insert(0, '/opt/trn_rl_repo')
import numpy as np
import ml_dtypes

import concourse.bass as bass
import concourse.bacc as bacc
import concourse.mybir as mybir
import concourse.tile as tile
from concourse.tile_rust import add_dep_helper
from concourse import bass_utils

BF16 = ml_dtypes.bfloat16

# model dims (hardcoded per spec)
B, S, D, H, KH, DH, L, V = 1, 2048, 2048, 16, 2, 128, 4, 10000
INNER = D * 4
EPS = 1e-5
NCORES = 8
SL = S // NCORES          # 256 tokens per core
KT = D // 128             # 16 d-ktiles
IT = INNER // 128         # 64 inner tiles
VT = 80                   # padded vocab tiles (80*128 = 10240)
VP = VT * 128
QK_OT = H + KH            # 18 output tiles for fused q|k projection
SCALE = DH ** -0.5
NJ = NCORES               # key blocks of 512 tokens (2 s-tiles each)

dt = mybir.dt
AF = mybir.ActivationFunctionType
OP = mybir.AluOpType

_cache = {}


def _build(n_layers, reps=1):
    nc = bacc.Bacc("TRN2", target_bir_lowering=False, debug=False,
                   enable_asserts=False, num_devices=NCORES)
    T = {}

    def inp(name, shape, d):
        T[name] = nc.dram_tensor(name, shape, d, kind="ExternalInput").ap()

    inp("x0T", [D, SL], dt.float32)
    inp("wqk", [n_layers * QK_OT * 128, KT * 128], dt.bfloat16)   # per-ot contraction-contiguous
    inp("wv", [n_layers * D, 2 * DH], dt.bfloat16)                # natural wv
    inp("wo_r", [n_layers * KT * 128, KT * 128], dt.bfloat16)
    inp("w1_r", [n_layers * IT * 128, KT * 128], dt.bfloat16)
    inp("w2_r", [n_layers * KT * 128, IT * 128], dt.bfloat16)
    inp("wh_r", [VT * 128, KT * 128], dt.bfloat16)
    inp("mg", [128, KT], dt.float32)
    inp("mb", [128, KT], dt.float32)
    inp("bg4", [n_layers * 128, KT], dt.float32)
    inp("bb4", [n_layers * 128, KT], dt.float32)
    inp("kn4", [n_layers * 128, 1], dt.float32)    # qn_g * kn_g folded product
    inp("qb4", [n_layers * 128, H], dt.float32)    # attn_b @ wq per-head bias cols
    inp("kb4", [n_layers * 128, KH], dt.float32)
    inp("vb4", [n_layers, 2 * DH], dt.float32)
    inp("b14", [n_layers * 128, IT], dt.float32)
    inp("b24", [n_layers * 128, KT], dt.float32)
    inp("hbias", [128, VT], dt.float32)
    inp("logm", [128, NJ], dt.float32)             # per-j exp bias: 0 visible / -30000 hidden/own
    inp("dmask", [128, 2 * SL], dt.bfloat16)       # relative causal mask for own block
    T["out"] = nc.dram_tensor("logitsT", [VP, SL], dt.float32, kind="ExternalOutput").ap()
    T["kv_in"] = [nc.dram_tensor(f"kv_in{l}", [512, SL], dt.bfloat16, kind="Internal").ap()
                  for l in range(n_layers)]
    T["kv_all"] = [nc.dram_tensor(f"kv_all{l}", [NCORES * 512, SL], dt.bfloat16,
                                  kind="Internal", addr_space="Shared").ap()
                   for l in range(n_layers)]

    with tile.TileContext(nc) as tc:
        for _ in range(reps):
            _kbody(nc, tc, T, n_layers)
    nc.compile()
    return nc


def _kbody(nc, tc, T, n_layers):
    import contextlib
    es = contextlib.ExitStack()
    const = es.enter_context(tc.tile_pool(name="const", bufs=1))
    parp = es.enter_context(tc.tile_pool(name="parp", bufs=4))    # per-layer params
    res = es.enter_context(tc.tile_pool(name="res", bufs=1))      # residual f32 streams
    act = es.enter_context(tc.tile_pool(name="act", bufs=2))      # transient tiles
    ab1 = es.enter_context(tc.tile_pool(name="ab1", bufs=1))      # per-layer bf16 sets
    wp = es.enter_context(tc.tile_pool(name="wp", bufs=3))        # 3-ot weight slabs
    wp2 = es.enter_context(tc.tile_pool(name="wp2", bufs=2))      # w2 full slabs
    kvp = es.enter_context(tc.tile_pool(name="kvp", bufs=1))      # resident kv (2MB)
    rowp = es.enter_context(tc.tile_pool(name="rowp", bufs=1))
    bcp = es.enter_context(tc.tile_pool(name="bcp", bufs=1))
    gpl = es.enter_context(tc.tile_pool(name="gpl", bufs=1))      # gelu out, resident
    lgp = es.enter_context(tc.tile_pool(name="lgp", bufs=2))      # head logits staging
    pmm = es.enter_context(tc.tile_pool(name="pmm", bufs=2, space="PSUM"))
    psc = es.enter_context(tc.tile_pool(name="psc", bufs=3, space="PSUM"))
    pao = es.enter_context(tc.tile_pool(name="pao", bufs=1, space="PSUM"))
    prw = es.enter_context(tc.tile_pool(name="prw", bufs=2, space="PSUM"))

    _last_slab = [None]

    def dma_seq(dst, src_ap):
        return nc.sync.dma_start(dst, src_ap)

    def dma_slab(dst, src_ap):
        return nc.sync.dma_start(dst, src_ap)

    ones = const.tile([128, 1], dt.bfloat16, tag="ones", name="ones")
    nc.gpsimd.memset(ones[:], 1.0)
    onesD = const.tile([128, 1], dt.bfloat16, tag="onesD", name="onesD")
    nc.gpsimd.memset(onesD[:], 1.0 / D)
    onesH = const.tile([128, 1], dt.bfloat16, tag="onesH", name="onesH")
    nc.gpsimd.memset(onesH[:], 1.0 / DH)

    logm_t = const.tile([128, NJ], dt.float32, tag="logm", name="logm")
    dma_seq(logm_t[:], T["logm"][:, :])
    dmask_t = const.tile([128, 2 * SL], dt.bfloat16, tag="dmask", name="dmask")
    dma_seq(dmask_t[:], T["dmask"][:, :])

    def loadc(pool, key, shape, tag, rows=None):
        t = pool.tile(shape, dt.float32, tag=tag)
        dma_seq(t[:], T[key] if rows is None else T[key][rows[0]:rows[1], :])
        return t

    mg_t = loadc(const, "mg", [128, KT], "mg")
    mb_t = loadc(const, "mb", [128, KT], "mb")
    hbias_t = loadc(const, "hbias", [128, VT], "hbias")

    last_sqrt = [None]

    def stat_rows(pr):
        """rows [rsd | -mean*rsd] from accumulated [mean|ex2] psum rows."""
        t = rowp.tile([1, 2 * SL], dt.float32, tag="r_t", name="r_t")
        nc.vector.tensor_copy(t[:], pr[:])
        m2 = rowp.tile([1, SL], dt.float32, tag="r_m2", name="r_m2")
        nc.vector.tensor_tensor(m2[:], t[:, 0:SL], t[:, 0:SL], OP.mult)
        varp = rowp.tile([1, SL], dt.float32, tag="r_var", name="r_var")
        nc.vector.scalar_tensor_tensor(varp[:], t[:, SL:2 * SL], EPS, m2[:],
                                       OP.add, OP.subtract)
        rec = rowp.tile([1, SL], dt.float32, tag="r_rec", name="r_rec")
        nc.vector.reciprocal(rec[:], varp[:])
        sm = rowp.tile([1, 2 * SL], dt.float32, tag="r_sm", name="r_sm")
        last_sqrt[0] = nc.scalar.activation(sm[:, 0:SL], rec[:], AF.Sqrt)
        nc.vector.scalar_tensor_tensor(sm[:, SL:2 * SL], t[:, 0:SL], -1.0, sm[:, 0:SL],
                                       OP.mult, OP.mult)
        return sm

    def stats_pass(xs):
        """layernorm stats over D (partition dim across KT tiles) -> bcast [128,2SL]."""
        pr = prw.tile([1, 2 * SL], dt.float32, tag="pr", name="pr")
        for kt in range(KT):
            sq = act.tile([128, 2 * SL], dt.bfloat16, tag="st_sq", name="st_sq")
            nc.vector.tensor_copy(sq[:, 0:SL], xs[kt][:])
            nc.vector.tensor_tensor(sq[:, SL:2 * SL], sq[:, 0:SL], sq[:, 0:SL], OP.mult)
            nc.tensor.matmul(pr[:], onesD[:], sq[:], start=(kt == 0), stop=(kt == KT - 1))
        sm = stat_rows(pr)
        bct = bcp.tile([128, 2 * SL], dt.float32, tag="bc_n", name="bc_n")
        nc.gpsimd.partition_broadcast(bct[:], sm[:])
        return bct

    def norm_apply_gb_stats(xs, bct, g_t, b_t, out_tag, out_pool):
        """apply norm and emit the output's own stats pass per-tile, so the
        stats matmuls pipeline behind the applies instead of after all 16."""
        outs = []
        pr = prw.tile([1, 2 * SL], dt.float32, tag="pr", name="pr")
        for kt in range(KT):
            cb = act.tile([128, SL], dt.float32, tag="ap_cb", name="ap_cb")
            nc.vector.tensor_scalar(cb[:], bct[:, SL:2 * SL], g_t[:, kt:kt + 1],
                                    b_t[:, kt:kt + 1], OP.mult, OP.add)
            t1 = act.tile([128, SL], dt.float32, tag="ap_t1", name="ap_t1")
            nc.vector.scalar_tensor_tensor(t1[:], xs[kt][:], g_t[:, kt:kt + 1],
                                           bct[:, 0:SL], OP.mult, OP.mult)
            y = out_pool.tile([128, SL], dt.float32, tag=f"{out_tag}{kt}",
                              name=f"{out_tag}{kt}")
            nc.vector.tensor_tensor(y[:], t1[:], cb[:], OP.add)
            sq = act.tile([128, 2 * SL], dt.bfloat16, tag="st_sq", name="st_sq")
            nc.vector.tensor_copy(sq[:, 0:SL], y[:])
            nc.vector.tensor_tensor(sq[:, SL:2 * SL], sq[:, 0:SL], sq[:, 0:SL], OP.mult)
            nc.tensor.matmul(pr[:], onesD[:], sq[:], start=(kt == 0), stop=(kt == KT - 1))
            outs.append(y)
        sm = stat_rows(pr)
        bct2 = bcp.tile([128, 2 * SL], dt.float32, tag="bc_n2", name="bc_n2")
        nc.gpsimd.partition_broadcast(bct2[:], sm[:])
        return outs, bct2

    def norm_apply_gb(xs, bct, g_t, b_t, out_tag, out_pool):
        outs = []
        for kt in range(KT):
            cb = act.tile([128, SL], dt.float32, tag="ap_cb", name="ap_cb")
            nc.vector.tensor_scalar(cb[:], bct[:, SL:2 * SL], g_t[:, kt:kt + 1],
                                    b_t[:, kt:kt + 1], OP.mult, OP.add)
            t1 = act.tile([128, SL], dt.float32, tag="ap_t1", name="ap_t1")
            nc.vector.scalar_tensor_tensor(t1[:], xs[kt][:], g_t[:, kt:kt + 1],
                                           bct[:, 0:SL], OP.mult, OP.mult)
            y = out_pool.tile([128, SL], dt.float32, tag=f"{out_tag}{kt}",
                              name=f"{out_tag}{kt}")
            nc.vector.tensor_tensor(y[:], t1[:], cb[:], OP.add)
            outs.append(y)
        return outs

    def norm_apply_plain(xs, bct, out_tag, out_pool, out_dt=dt.bfloat16):
        outs = []
        for kt in range(KT):
            t1 = act.tile([128, SL], dt.float32, tag="ap_t1", name="ap_t1")
            nc.vector.tensor_tensor(t1[:], xs[kt][:], bct[:, 0:SL], OP.mult)
            y = out_pool.tile([128, SL], out_dt, tag=f"{out_tag}{kt}",
                              name=f"{out_tag}{kt}")
            nc.vector.tensor_tensor(y[:], t1[:], bct[:, SL:2 * SL], OP.add)
            outs.append(y)
        return outs

    def qk_norm(ps, bias_col, g_ap, out_tag, out_pool):
        """qk-norm of psum [128(dh), SL] (+per-dh bias); stats over dh."""
        qb = act.tile([128, 2 * SL], dt.bfloat16, tag="qk_sq", name="qk_sq")
        nc.vector.tensor_scalar_add(qb[:, 0:SL], ps[:], bias_col)
        nc.vector.tensor_tensor(qb[:, SL:2 * SL], qb[:, 0:SL], qb[:, 0:SL], OP.mult)
        pr = prw.tile([1, 2 * SL], dt.float32, tag="pr", name="pr")
        nc.tensor.matmul(pr[:], onesH[:], qb[:], start=True, stop=True)
        sm = stat_rows(pr)
        bct = bcp.tile([128, 2 * SL], dt.float32, tag="bc_qk", name="bc_qk")
        nc.gpsimd.partition_broadcast(bct[:], sm[:])
        if g_ap is not None:
            nc.vector.tensor_scalar_mul(bct[:], bct[:], g_ap)
        t1 = act.tile([128, SL], dt.float32, tag="qk_t1", name="qk_t1")
        nc.vector.tensor_tensor(t1[:], qb[:, 0:SL], bct[:, 0:SL], OP.mult)
        y = out_pool.tile([128, SL], dt.bfloat16, tag=out_tag)
        nc.vector.tensor_tensor(y[:], t1[:], bct[:, SL:2 * SL], OP.add)
        return y

    def proj_chunks(wkey, ot0, n_ot, rhs, consume, dep_from=None, pool=None):
        """projection group: batched 4-ot slabs, one accumulation chain per ot.

        dep_from: scheduler-level ordering edge added to each chain's first
        matmul, so a consumer stage can never capture all psum slots while its
        producer stage still needs one (slot-capture deadlock).
        """
        c0 = 0
        last_mm = dep_from
        while c0 < n_ot:
            cn = min(3, n_ot - c0)
            slab = wp.tile([128, 3 * KT * 128], dt.bfloat16, tag="w", name="w")
            src = T[wkey][(ot0 + c0) * 128:(ot0 + c0 + cn) * 128, :]
            dma_slab(slab[:, 0:cn * KT * 128],
                     src.rearrange("(o p) c -> p o c", p=128))
            for o in range(cn):
                pl, ptag = (pmm, "mm") if pool is None else pool
                ps = pl.tile([128, SL], dt.float32, tag=ptag, name=ptag)
                for kt in range(KT):
                    mm = nc.tensor.matmul(ps[:], slab[:, o * 2048 + kt * 128:o * 2048 + (kt + 1) * 128],
                                          rhs[kt][:], start=(kt == 0), stop=(kt == KT - 1))
                    if kt == 0 and last_mm is not None:
                        add_dep_helper(last_mm.ins, mm.ins,
                                       reason="chain order: slots release in program order")
                    last_mm = mm
                consume(c0 + o, ps)
            c0 += cn
        return last_mm

    # ---------- x = model_norm(x0T) ----------
    x0 = []
    for kt in range(KT):
        t = res.tile([128, SL], dt.float32, tag=f"rx{kt}", name=f"rx{kt}")
        dma_seq(t[:], T["x0T"][kt * 128:(kt + 1) * 128, :])
        x0.append(t)
    xs, bct_x = norm_apply_gb_stats(x0, stats_pass(x0), mg_t, mb_t, "rx", res)

    for l in range(n_layers):
        bg = loadc(parp, "bg4", [128, KT], "bg", (l * 128, (l + 1) * 128))
        bb = loadc(parp, "bb4", [128, KT], "bb", (l * 128, (l + 1) * 128))
        kn = parp.tile([128, 1], dt.float32, tag="kn", name="kn")
        dma_seq(kn[:], T["kn4"][l * 128:(l + 1) * 128, :])
        qbias = loadc(parp, "qb4", [128, H], "qbt", (l * 128, (l + 1) * 128))
        kbias = loadc(parp, "kb4", [128, KH], "kbt", (l * 128, (l + 1) * 128))
        vbrow = parp.tile([1, 2 * DH], dt.float32, tag="vbr", name="vbr")
        dma_seq(vbrow[:], T["vb4"][l:l + 1, :])
        vbias = bcp.tile([128, 2 * DH], dt.float32, tag="bc_vb", name="bc_vb")
        nc.gpsimd.partition_broadcast(vbias[:], vbrow[:])
        b1t = loadc(parp, "b14", [128, IT], "b1", (l * 128, (l + 1) * 128))
        b2t = loadc(parp, "b24", [128, KT], "b2", (l * 128, (l + 1) * 128))

        hs, bct_h = norm_apply_gb_stats(xs, bct_x, bg, bb, "rh", res)
        hn = norm_apply_plain(hs, bct_h, "hnb", ab1)

        # ---- k, v first (feed collective) ----
        kfin = [None] * KH
        def kcons(o, ps):
            kf = qk_norm(ps, kbias[:, o:o + 1], kn[:, 0:1], f"kf{o}", ab1)
            kfin[o] = kf
            dma_seq(T["kv_in"][l][o * 128:(o + 1) * 128, :], kf[:])
        k_last = proj_chunks("wqk", l * QK_OT + H, KH, hn, kcons)

        # v natural: lhsT = hn tok-slice, rhs = wv slab
        vps = [pmm.tile([128, 2 * DH], dt.float32, tag="mm", name="mm") for _ in range(2)]
        v_last = k_last
        for c in range(2):
            wvs = wp.tile([128, 8 * 2 * DH], dt.bfloat16, tag="w", name="wv")
            src = T["wv"][l * D + c * 1024:l * D + (c + 1) * 1024, :]
            dma_slab(wvs[:], src.rearrange("(k p) d -> p k d", p=128))
            for k in range(8):
                kt = c * 8 + k
                for tt in range(2):
                    mm = nc.tensor.matmul(vps[tt][:], hn[kt][:, tt * 128:(tt + 1) * 128],
                                          wvs[:, k * 256:(k + 1) * 256],
                                          start=(kt == 0), stop=(kt == KT - 1))
                    if kt == 0:
                        add_dep_helper(v_last.ins, mm.ins,
                                       reason="chain order: slots release in program order")
            v_last = mm
        vbt = [None, None]
        for tt in range(2):
            vb = ab1.tile([128, 2 * DH], dt.bfloat16, tag=f"vb{tt}", name=f"vb{tt}")
            nc.vector.tensor_tensor(vb[:], vps[tt][:], vbias[:], OP.add)
            vbt[tt] = vb
            dma_seq(T["kv_in"][l][256 + tt * 128: 256 + (tt + 1) * 128, :], vb[:])

        nc.gpsimd.collective_compute(
            "AllGather", OP.bypass, replica_groups=[list(range(NCORES))],
            ins=[T["kv_in"][l]], outs=[T["kv_all"][l]])

        # ---- q projections + qk-norm (overlap the collective) ----
        qfin = [None] * H
        def qcons(o, ps):
            qfin[o] = qk_norm(ps, qbias[:, o:o + 1], None, f"qa{o}", ab1)
        q_last = proj_chunks("wqk", l * QK_OT, H, hn, qcons, dep_from=v_last)

        # ---- gathered kv -> per-block resident SBUF tiles (attention on block j
        # starts as soon as its own DMA lands, not after all 8) ----
        kvs = []
        for j in range(NJ):
            t = kvp.tile([128, 4 * SL], dt.bfloat16, tag=f"kv{j}", name=f"kv{j}")
            src = T["kv_all"][l][j * 512:(j + 1) * 512, :]
            dma_seq(t[:], src.rearrange("(r p) t -> p r t", p=128))
            kvs.append(t)

        # ---- attention: own diagonal block first (hides the collective), then gathered ----
        ao = []
        for qh in range(H):
            kh = qh // (H // KH)
            aops = pao.tile([128, SL], dt.float32, tag="ao", name="ao")
            den = psc.tile([1, SL], dt.float32, tag="sc", name="den")
            scl = psc.tile([128, 2 * SL], dt.float32, tag="sc", name="sc")
            for hf in range(2):
                nc.tensor.matmul(scl[:, hf * SL:(hf + 1) * SL],
                                 kfin[kh][:, hf * 128:(hf + 1) * 128], qfin[qh][:],
                                 start=True, stop=True)
            pTl = act.tile([128, 2 * SL], dt.bfloat16, tag="pT", name="pT")
            nc.scalar.activation(pTl[:], scl[:], AF.Exp, scale=SCALE)
            pTm = ab1.tile([128, 2 * SL], dt.bfloat16, tag="pTm", name="pTm")
            nc.vector.tensor_tensor(pTm[:], pTl[:], dmask_t[:], OP.mult)
            for hf in range(2):
                nc.tensor.matmul(den[:], ones[:], pTm[:, hf * SL:(hf + 1) * SL],
                                 start=(hf == 0), stop=False)
                nc.tensor.matmul(aops[:], vbt[hf][:, kh * 128:(kh + 1) * 128],
                                 pTm[:, hf * SL:(hf + 1) * SL],
                                 start=(hf == 0), stop=False)
            for j in range(NJ):
                sc = psc.tile([128, 2 * SL], dt.float32, tag="sc", name="sc")
                for hf in range(2):
                    kof = kh * SL + hf * 128
                    nc.tensor.matmul(sc[:, hf * SL:(hf + 1) * SL],
                                     kvs[j][:, kof:kof + 128], qfin[qh][:],
                                     start=True, stop=True)
                pT = act.tile([128, 2 * SL], dt.bfloat16, tag="pT", name="pT")
                nc.scalar.activation(pT[:], sc[:], AF.Exp, scale=SCALE,
                                     bias=logm_t[:, j:j + 1])
                for hf in range(2):
                    nc.tensor.matmul(den[:], ones[:], pT[:, hf * SL:(hf + 1) * SL],
                                     start=False, stop=(j == NJ - 1 and hf == 1))
                    vof = (2 + hf) * SL + kh * 128
                    nc.tensor.matmul(aops[:], kvs[j][:, vof:vof + 128],
                                     pT[:, hf * SL:(hf + 1) * SL],
                                     start=False, stop=(j == NJ - 1 and hf == 1))
            recd = rowp.tile([1, SL], dt.float32, tag="r_recd", name="r_recd")
            nc.vector.reciprocal(recd[:], den[:])
            rb = bcp.tile([128, SL], dt.float32, tag="bc_den", name="bc_den")
            nc.gpsimd.partition_broadcast(rb[:], recd[:])
            aot = ab1.tile([128, SL], dt.bfloat16, tag=f"hnb{qh}", name=f"ao{qh}")
            nc.vector.tensor_tensor(aot[:], aops[:], rb[:], OP.mult)
            ao.append(aot)

        # ---- wo + residual (h2 reuses the rh tag slots) ----
        h2 = [None] * KT
        h2b = [None] * KT
        def ocons(o, ps):
            t = res.tile([128, SL], dt.float32, tag=f"r2{o}", name=f"r2{o}")
            nc.vector.tensor_tensor(t[:], ps[:], hs[o][:], OP.add)
            h2[o] = t
            tb = ab1.tile([128, SL], dt.bfloat16, tag=f"hnb{o}", name=f"h2b{o}")
            nc.vector.tensor_copy(tb[:], t[:])
            h2b[o] = tb
        wo_last = proj_chunks("wo_r", l * KT, KT, ao, ocons, dep_from=q_last)

        # ---- mlp ----
        gts = [None] * IT
        def gcons(o, ps):
            gt = gpl.tile([128, SL], dt.bfloat16, tag=f"g{o}", name=f"g{o}")
            nc.scalar.activation(gt[:], ps[:], AF.Gelu_apprx_tanh, bias=b1t[:, o:o + 1])
            gts[o] = gt
        w1_last = proj_chunks("w1_r", l * IT, IT, h2b, gcons, dep_from=wo_last)

        xs = []
        w2_prev = w1_last
        for ot in range(KT):
            ps = pmm.tile([128, SL], dt.float32, tag="mm", name="mm")
            for hf in range(2):
                slab = wp2.tile([128, (IT // 2) * 128], dt.bfloat16, tag="w2", name="w2")
                dma_slab(slab[:], T["w2_r"][(l * KT + ot) * 128:(l * KT + ot + 1) * 128,
                                            hf * 4096:(hf + 1) * 4096])
                for k in range(IT // 2):
                    it = hf * (IT // 2) + k
                    mm = nc.tensor.matmul(ps[:], slab[:, k * 128:(k + 1) * 128], gts[it][:],
                                          start=(it == 0), stop=(it == IT - 1))
                    if it == 0:
                        add_dep_helper(w2_prev.ins, mm.ins,
                                       reason="chain order: slots release in program order")
                    w2_prev = mm
            t = res.tile([128, SL], dt.float32, tag=f"rx{ot}", name=f"rx{ot}")
            nc.vector.scalar_tensor_tensor(t[:], ps[:], b2t[:, ot:ot + 1], h2[ot][:],
                                           OP.add, OP.add)
            xs.append(t)
        xs, bct_x = norm_apply_gb_stats(xs, stats_pass(xs), mg_t, mb_t, "rx", res)

    # ---------- head (norm folded into wh/hbias) ----------
    xh = norm_apply_plain(xs, bct_x, "g", gpl)
    lgs = {}
    def hcons(o, ps):
        c, o2 = o // 2, o % 2
        if o2 == 0:
            lgs[c] = lgp.tile([128, 2 * SL], dt.float32, tag="lg", name="lg")
        nc.vector.tensor_scalar_add(lgs[c][:, o2 * SL:(o2 + 1) * SL], ps[:],
                                    hbias_t[:, o:o + 1])
        if o2 == 1:
            dst = T["out"][c * 256:(c + 1) * 256, :]
            dma_seq(dst.rearrange("(o p) t -> p o t", p=128), lgs[c][:])
    proj_chunks("wh_r", 0, VT, xh, hcons)
    es.close()


# ---------------- host side ----------------

def _rearrange_w(wl, n_out):
    """[D_in, n_out*128] -> [n_out*128, KT*128] contraction-contiguous blocks."""
    d_in = wl.shape[0]
    nk = d_in // 128
    return np.ascontiguousarray(
        wl.reshape(nk, 128, n_out, 128).transpose(2, 1, 0, 3).reshape(n_out * 128, nk * 128))


def _prep_inputs(inputs, n_layers):
    text = np.asarray(inputs["text"]).reshape(S)
    embed_w = np.asarray(inputs["embed_w"], dtype=np.float32)
    ag = np.asarray(inputs["attn_norm_g"], dtype=np.float32)
    ab = np.asarray(inputs["attn_norm_b"], dtype=np.float32)
    wq = np.asarray(inputs["wq"], dtype=np.float32)
    wk = np.asarray(inputs["wk"], dtype=np.float32)
    wv = np.asarray(inputs["wv"], dtype=np.float32)
    wo = np.asarray(inputs["wo"], dtype=np.float32)
    w1 = np.asarray(inputs["w1"], dtype=np.float32)
    w2 = np.asarray(inputs["w2"], dtype=np.float32)
    qn = np.asarray(inputs["qn_g"], dtype=np.float32)
    knv = np.asarray(inputs["kn_g"], dtype=np.float32)
    head_w = np.asarray(inputs["head_w"], dtype=np.float32)
    hg = np.asarray(inputs["head_norm_g"], dtype=np.float32)
    hb = np.asarray(inputs["head_norm_b"], dtype=np.float32)

    def c16(x):
        return np.ascontiguousarray(x.astype(BF16))

    wqk_l, wv_l, wo_l, w1_l, w2_l = [], [], [], [], []
    qb_l, kb_l, vb_l = [], [], []
    for l in range(n_layers):
        # fold attn prenorm gain into wq/wk/wv; bias -> additive projections
        wq_f = ag[l][:, None] * wq[l]
        wk_f = ag[l][:, None] * wk[l]
        wv_f = ag[l][:, None] * wv[l]
        qb_l.append(ab[l] @ wq[l])              # [H*DH]
        kb_l.append(ab[l] @ wk[l])              # [KH*DH]
        vb_l.append(ab[l] @ wv[l])              # [KH*DH]
        qk = np.concatenate([wq_f, wk_f], axis=1)
        wqk_l.append(_rearrange_w(qk, QK_OT))
        wv_l.append(wv_f)
        wo_l.append(_rearrange_w(wo[l], KT))
        w1_l.append(_rearrange_w(w1[l], IT))
        w2_l.append(_rearrange_w(w2[l], KT))
    # fold head norm into head weight + bias
    wh_f = hg[:, None] * head_w
    hbias_f = np.asarray(inputs["head_b"], np.float32) + hb @ head_w
    wh_pad = np.zeros((D, VP), np.float32)
    wh_pad[:, :V] = wh_f
    wh_r = _rearrange_w(wh_pad, VT)

    def ncol(v, nt=KT):  # [n_layers, nt*128] -> [n_layers*128, nt]
        return np.ascontiguousarray(
            np.asarray(v, np.float32)[:n_layers].reshape(n_layers, nt, 128)
            .transpose(0, 2, 1).reshape(n_layers * 128, nt))

    def ncol1(v, nk):
        return np.ascontiguousarray(
            np.asarray(v, np.float32).reshape(nk, 128).transpose(1, 0))

    shared = {
        "wqk": c16(np.concatenate(wqk_l, axis=0)),
        "wv": c16(np.concatenate(wv_l, axis=0)),
        "wo_r": c16(np.concatenate(wo_l, axis=0)),
        "w1_r": c16(np.concatenate(w1_l, axis=0)),
        "w2_r": c16(np.concatenate(w2_l, axis=0)),
        "wh_r": c16(wh_r),
        "mg": ncol1(inputs["model_norm_g"], KT),
        "mb": ncol1(inputs["model_norm_b"], KT),
        "bg4": ncol(inputs["blk_norm_g"]),
        "bb4": ncol(inputs["blk_norm_b"]),
        "kn4": np.ascontiguousarray((qn * knv)[:n_layers].reshape(n_layers * 128, 1)),
        "qb4": np.ascontiguousarray(
            np.stack(qb_l).reshape(n_layers, H, 128).transpose(0, 2, 1)
            .reshape(n_layers * 128, H)),
        "kb4": np.ascontiguousarray(
            np.stack(kb_l).reshape(n_layers, KH, 128).transpose(0, 2, 1)
            .reshape(n_layers * 128, KH)),
        "vb4": np.ascontiguousarray(np.stack(vb_l)),
        "b14": ncol(inputs["b1"], IT),
        "b24": ncol(inputs["b2"]),
        "hbias": ncol1(np.concatenate([hbias_f, np.zeros(VP - V, np.float32)]), VT),
    }

    # relative causal mask for the own (diagonal) 512-token block: same for all cores
    p = np.arange(128)
    t = np.arange(SL)
    dmask = np.concatenate([(p[:, None] <= t[None, :]), (128 + p[:, None] <= t[None, :])],
                           axis=1).astype(BF16)                  # [128, 2*SL]
    in_maps = []
    for c in range(NCORES):
        toks = text[c * SL:(c + 1) * SL]
        x0T = np.ascontiguousarray(embed_w[toks].T.astype(np.float32))
        logm = np.zeros((128, NJ), np.float32)
        logm[:, c:] = -30000.0          # own block handled locally; later blocks hidden
        im = dict(shared)
        im["x0T"] = x0T
        im["logm"] = logm
        im["dmask"] = np.ascontiguousarray(dmask)
        in_maps.append(im)
    return in_maps


def _get_nc(n_layers):
    import os
    reps = int(os.environ.get("KERNEL_REPS", "1"))
    key = ("nc", n_layers, reps)
    if key not in _cache:
        _cache[key] = _build(n_layers, reps)
    return _cache[key]


def kernel(**inputs):
    return run(inputs, L)[0]


def run(inputs, n_layers, trace=False):
    nc = _get_nc(n_layers)
    in_maps = _prep_inputs(inputs, n_layers)
    res = bass_utils.run_bass_kernel_spmd(nc, in_maps, core_ids=list(range(NCORES)),
                                          trace=trace)
    parts = [res.results[c]["logitsT"][:V, :].T for c in range(NCORES)]
    logits = np.concatenate(parts, axis=0).reshape(B, S, V).astype(np.float32)
    return logits, res


def _make_runner(nc):
    import jax
    from jax.experimental.shard_map import shard_map
    from jax.sharding import Mesh, PartitionSpec
    from concourse import bass2jax as b2j
    b2j.install_neuronx_cc_hook()
    partition_name = nc.partition_id_tensor.name if nc.partition_id_tensor else None
    in_names, out_names, out_avals, zero_outs = [], [], [], []
    for alloc in nc.m.functions[0].allocations:
        if not isinstance(alloc, mybir.MemoryLocationSet):
            continue
        name = alloc.memorylocations[0].name
        if alloc.kind == "ExternalInput":
            if name != partition_name:
                in_names.append(name)
        elif alloc.kind == "ExternalOutput":
            shape = tuple(alloc.tensor_shape)
            d = mybir.dt.np(alloc.dtype)
            out_names.append(name)
            out_avals.append(jax.core.ShapedArray(shape, d))
            zero_outs.append(np.zeros(shape, d))
    n_params = len(in_names)
    all_names = in_names + out_names
    if partition_name is not None:
        all_names.append(partition_name)

    def _body(*args):
        operands = list(args)
        if partition_name is not None:
            operands.append(b2j.partition_id_tensor())
        outs = b2j._bass_exec_p.bind(
            *operands, out_avals=tuple(out_avals), in_names=tuple(all_names),
            out_names=tuple(out_names), lowering_input_output_aliases=(),
            sim_require_finite=False, sim_require_nnan=False, nc=nc)
        return tuple(outs)

    devices = jax.devices()[:NCORES]
    mesh = Mesh(np.asarray(devices), ("core",))
    n_in = n_params + len(out_names)
    sharded = jax.jit(
        shard_map(_body, mesh=mesh, in_specs=(PartitionSpec("core"),) * n_in,
                  out_specs=(PartitionSpec("core"),) * len(out_names), check_rep=False),
        keep_unused=True)
    from jax.sharding import NamedSharding
    shspec = NamedSharding(mesh, PartitionSpec("core"))
    return dict(fn=sharded, in_names=in_names, out_names=out_names,
                zero_outs=zero_outs, n_params=n_params, shspec=shspec)


def run_timed(inputs, n_layers, iters=3):
    import jax, time
    nc = _get_nc(n_layers)
    key = ("runner", n_layers)
    if key not in _cache:
        _cache[key] = _make_runner(nc)
    R = _cache[key]
    in_maps = _prep_inputs(inputs, n_layers)
    concat_in = [np.concatenate([np.asarray(in_maps[c][nm]) for c in range(NCORES)], axis=0)
                 for nm in R["in_names"]]
    concat_zero = [np.zeros((NCORES * z.shape[0], *z.shape[1:]), z.dtype)
                   for z in R["zero_outs"]]
    args = [jax.device_put(a, R["shspec"]) for a in concat_in + concat_zero]
    for a in args:
        a.block_until_ready()
    t0 = time.time()
    outs = R["fn"](*args)
    [o.block_until_ready() for o in outs]
    t1 = time.time()
    times = [t1 - t0]
    for _ in range(iters - 1):
        time.sleep(0.05)
        t0 = time.time()
        outs = R["fn"](*args)
        [o.block_until_ready() for o in outs]
        times.append(time.time() - t0)
    lt = np.asarray(outs[R["out_names"].index("logitsT")]).reshape(NCORES, VP, SL)
    parts = [lt[c][:V, :].T for c in range(NCORES)]
    logits = np.concatenate(parts, axis=0).reshape(B, S, V).astype(np.float32)
    return logits, times
